# revision 27
# baseline (speedup 1.0000x reference)
"""Trainium2 Bass kernel for nn_ConditionalSoftmax (sampled-softmax NLL loss).

Computes, for each batch row b:
    v_c   = vectors[cs[b]]                      # [D]
    h     = relu(v_c @ W1 + b1)                 # [H]
    logit = h @ W2 + b2                         # [V]
    nll_b = logsumexp(logit) - logit[v2s[ws[b]]]

Sharding: data-parallel over batch across 8 NeuronCores (1024 rows/core),
weights replicated.  Per core the dominant work is the [1024,512]@[512,20000]
matmul plus the exp of all 20.5M logits.  The matmul runs in fp8_e4m3 with
the PE's DoubleRow perf mode (K=256 per instruction, 2x bf16 throughput);
W2 is pre-scaled by 32 on the host so its values sit in the fp8 normal
range, and the Exp activation's scale parameter undoes the factor for free.
W2 (fp8, 80KB/partition) stays resident in SBUF.  Logits accumulate in
[128,2000] PSUM tiles (4 banks, double buffered) and are reduced in place
by the ScalarEngine's fused exp+row-sum (accum_out), so the [1024,20000]
logit matrix never touches HBM and the per-instruction activation overhead
is amortized over 2000 columns.  The final log runs as ONE batched Ln over
[128,8] so the Exp/Ln activation tables swap exactly once.  The target
logit takes a separate cheap path: indirect-gather of the needed W2.T rows
(fp32) and a multiply-reduce on the VectorEngine against an fp32 recompute
of h.
"""

import numpy as np
import ml_dtypes

import concourse.bass as bass
import concourse.mybir as mybir
import concourse.tile as tile
from concourse import bacc, bass_utils
from concourse.bass import IndirectOffsetOnAxis, ts

# Problem shapes (hardcoded per contest contract)
N_VOCAB = 50000
V = 20000
D = 300
DP = 384          # D padded to 3*128
NDC = 3           # contraction chunks for D
H = 512
NKG = 2           # DoubleRow contraction groups for H (256 each)
NHC = 4           # 128-row contraction chunks for H
B = 8192
NCORES = 8
BL = B // NCORES  # 1024 rows per core
NBT = BL // 128   # 8 batch tiles of 128 rows
VT = 2000         # vocab tile width (4 PSUM banks)
NVT = V // VT     # 10 vocab tiles
# matmul chunks within a VT tile (cannot cross a 512-f32 PSUM bank boundary)
VCHUNKS = ((0, 512), (512, 512), (1024, 512), (1536, VT - 1536))

W2_SCALE = 32.0   # host pre-scale of W2 into fp8 range; undone by Exp scale

F32 = mybir.dt.float32
BF16 = mybir.dt.bfloat16
FP8 = mybir.dt.float8e4
I32 = mybir.dt.int32
AF = mybir.ActivationFunctionType
OP = mybir.AluOpType
DR = mybir.MatmulPerfMode.DoubleRow

_BUILD_CACHE = {}


def _build(b1_nz: bool, b2_nz: bool):
    key = (b1_nz, b2_nz)
    if key in _BUILD_CACHE:
        return _BUILD_CACHE[key]

    nc = bacc.Bacc(
        "TRN2",
        target_bir_lowering=False,
        debug=False,
        num_devices=NCORES,
        num_swdge_queues=4,
    )

    # Index tensors pre-transposed on the host to [128, NBT] so each loads
    # in ONE cheap DMA (contiguous 32B runs per partition).  ws is already
    # mapped through vector_to_support on the host.
    cs_idx = nc.dram_tensor("cs_idx", [128, NBT], I32, kind="ExternalInput").ap()
    ws_sup = nc.dram_tensor("ws_sup", [128, NBT], I32, kind="ExternalInput").ap()
    vectors = nc.dram_tensor("vectors", [N_VOCAB, D], F32, kind="ExternalInput").ap()
    w1 = nc.dram_tensor("w1", [DP, H], BF16, kind="ExternalInput").ap()
    b1c = nc.dram_tensor("b1c", [128, NHC], F32, kind="ExternalInput").ap()
    # W2 pre-scaled by W2_SCALE, fp8, laid out [v-chunk, p, kg, i, vt] with
    # W2s[kg*256 + i*128 + p, v*VT + vt] so each v-chunk is one fully
    # contiguous 1MB DRAM block (minimal DMA descriptor count — the early
    # window is descriptor-processing-bound across all 16 DMA queues).
    w2q = nc.dram_tensor(
        "w2q", [NVT, 128, NKG, 2, VT], FP8, kind="ExternalInput"
    ).ap()
    w2tb = nc.dram_tensor("w2tb", [V, H + 1], F32, kind="ExternalInput").ap()
    if b1_nz:
        b1rep = nc.dram_tensor("b1rep", [128, H], F32, kind="ExternalInput").ap()
    if b2_nz:
        b2rep = nc.dram_tensor("b2rep", [128, V], F32, kind="ExternalInput").ap()
    nll = nc.dram_tensor("nll", [128, NBT], F32, kind="ExternalOutput").ap()

    with tile.TileContext(nc) as tc:
        with (
            tc.tile_pool(name="consts", bufs=1) as consts,
            tc.tile_pool(name="idx", bufs=8) as idxp,
            tc.tile_pool(name="vc", bufs=8) as vcp,
            tc.tile_pool(name="gw", bufs=4) as gwp,
            tc.tile_pool(name="scr", bufs=2) as scrp,
            tc.tile_pool(name="acc", bufs=2) as accp,
            tc.tile_pool(name="ps", bufs=2, space="PSUM") as psm,
        ):
            # Index DMAs first: they gate the whole phase-1 chain and the
            # Sync sequencer issues DMAs serially (~600ns each).
            cidx = consts.tile([128, NBT], I32)
            nc.sync.dma_start(cidx[:], cs_idx[:])
            widx = consts.tile([128, NBT], I32)
            nc.sync.dma_start(widx[:], ws_sup[:])
            b1sb = consts.tile([128, NHC], F32)
            nc.sync.dma_start(b1sb[:], b1c[:])
            w1sb = consts.tile([128, NDC, H], BF16)
            nc.sync.dma_start(w1sb[:], w1.rearrange("(c p) h -> p c h", p=128))
            if b1_nz:
                b1rep_sb = consts.tile([128, H], F32)
                nc.sync.dma_start(b1rep_sb[:], b1rep[:])
            if b2_nz:
                b2rep_sb = consts.tile([128, V], F32)
                nc.sync.dma_start(b2rep_sb[:], b2rep[:])

            # Resident fp8 W2, loaded in v-chunks so phase 2 can start on
            # chunk 0 while later chunks stream in.  One tile per chunk so
            # dependency tracking is per-chunk, issued on Sync BEFORE the
            # transposes (HWDGE queues are FIFO: these must not sit behind
            # DMAs that wait on the slow gathers).  Keeping them out of the
            # Scalar stream matters even more: the Scalar sequencer is
            # in-order, and DMA flow-control waits there would stall every
            # Exp behind them.
            w2sbs = []
            for v in range(NVT):
                w2sb_v = consts.tile([128, NKG, 2, VT], FP8, name=f"w2sb{v}")
                nc.sync.dma_start(w2sb_v[:], w2q[v])
                w2sbs.append(w2sb_v)

            # Long-lived activations
            vcT = consts.tile([128, NDC, BL], BF16)    # v_c^T, d-major
            hT8 = consts.tile([128, NKG, 2, BL], FP8)  # h^T fp8, DoubleRow layout
            hb = consts.tile([128, NBT, H], F32)       # h, batch-major (target dot)
            tdot = consts.tile([128, NBT], F32)        # target logits
            fin = consts.tile([128, 3 * NBT], F32)     # S | lnS | result

            # ---- Phase helpers, software-pipelined into the phase-2 sweep
            # so no engine's in-order stream ever bunches slow work in front
            # of the critical path. ----

            def gather(t):
                """vc embedding gather for batch tile t (gpsimd)."""
                vc = vcp.tile([128, D], F32, tag="vc", name=f"vc{t}")
                nc.gpsimd.indirect_dma_start(
                    out=vc[:],
                    out_offset=None,
                    in_=vectors[:],
                    in_offset=IndirectOffsetOnAxis(ap=cidx[:, t : t + 1], axis=0),
                )
                return vc

            def cast(t, vc):
                """bf16 cast for batch tile t."""
                vcb = vcp.tile([128, DP], BF16, tag="vcb", name=f"vcb{t}")
                nc.vector.memset(vcb[:, D:DP], 0.0)
                nc.vector.tensor_copy(vcb[:, :D], vc[:])
                return vcb

            def transpose(t, vcb):
                """XBAR transposes for batch tile t.  Issued from the Scalar
                engine's HWDGE queues: the Sync queues carry the 1MB W2
                chunks, whose slow completions fill the ring and would stall
                these small critical DMAs behind them."""
                for c in range(NDC):
                    nc.scalar.dma_start(
                        vcT[:, c, ts(t, 128)], vcb[:, ts(c, 128)], transpose=True
                    )

            def first_layer_hT(t):
                """W1 matmuls + relu + fp8 cast for batch tile t.  Each hc
                chunk is an independent accumulation group, and start=True
                zeroes the whole 2KB PSUM bank, so each group gets its own
                bank (columns hc*512)."""
                pst = psm.tile([128, VT], F32, tag="ps", name=f"pst{t}")
                for hc in range(NHC):
                    for c in range(NDC):
                        nc.tensor.matmul(
                            pst[:, hc * 512 : hc * 512 + 128],
                            lhsT=w1sb[:, c, ts(hc, 128)],
                            rhs=vcT[:, c, ts(t, 128)],
                            start=(c == 0),
                            stop=(c == NDC - 1),
                        )
                # relu + bias, cast to fp8 DoubleRow layout (DVE)
                for hc in range(NHC):
                    nc.vector.tensor_scalar(
                        out=hT8[:, hc // 2, hc % 2, ts(t, 128)],
                        in0=pst[:, hc * 512 : hc * 512 + 128],
                        scalar1=b1sb[:, hc : hc + 1],
                        scalar2=0.0,
                        op0=OP.add,
                        op1=OP.max,
                    )

            def first_layer_hb(t):
                """Batch-major fp32 h for the target-logit dot."""
                psb = psm.tile([128, VT], F32, tag="ps", name=f"psb{t}")
                for c in range(NDC):
                    nc.tensor.matmul(
                        psb[:, :512],
                        lhsT=vcT[:, c, ts(t, 128)],
                        rhs=w1sb[:, c, :],
                        start=(c == 0),
                        stop=(c == NDC - 1),
                    )
                if b1_nz:
                    nc.vector.tensor_add(psb[:, :512], psb[:, :512], b1rep_sb[:])
                nc.vector.tensor_scalar_max(hb[:, t, :], psb[:, :512], 0.0)

            def target_dot(t):
                """Gather W2.T row of the target, fp32 dot with h."""
                g = gwp.tile([128, H + 1], F32, tag="g", name=f"g{t}")
                nc.gpsimd.indirect_dma_start(
                    out=g[:],
                    out_offset=None,
                    in_=w2tb[:],
                    in_offset=IndirectOffsetOnAxis(ap=widx[:, t : t + 1], axis=0),
                )
                # (tensor_tensor_reduce is broken on this HW path; use 3 ops)
                gscr = gwp.tile([128, H], F32, tag="gscr", name=f"gscr{t}")
                nc.vector.tensor_mul(gscr[:], hb[:, t, :], g[:, :H])
                gacc = gwp.tile([128, 1], F32, tag="gacc", name=f"gacc{t}")
                nc.vector.reduce_sum(
                    out=gacc[:], in_=gscr[:], axis=mybir.AxisListType.X
                )
                nc.vector.tensor_add(tdot[:, t : t + 1], gacc[:], g[:, H : H + 1])

            # ---- Main pipeline.  All gathers queue on gpsimd immediately
            # (they pace at several us each due to SWDGE ring flow control,
            # but nothing downstream needs tile t before its sweep).  Per
            # batch-tile sweep over v, the NEXT tile's cast/transpose,
            # first-layer and this tile's target-dot are injected at
            # staggered points so no in-order engine stream ever queues
            # slow work in front of critical work.  The exp row-sums run on
            # the otherwise-idle VectorEngine (bf16, 2x mode) so the Scalar
            # engine does nothing but back-to-back Exp. ----
            vcs = [gather(t) for t in range(NBT)]
            transpose(0, cast(0, vcs[0]))
            first_layer_hT(0)
            first_layer_hb(0)
            vcbs = {}
            for t in range(NBT):
                acc = accp.tile([128, VT], BF16, tag="acc", name=f"acc{t}")
                for v in range(NVT):
                    if v == 2 and t + 1 < NBT:
                        vcbs[t + 1] = cast(t + 1, vcs[t + 1])
                    if v == 4 and t + 1 < NBT:
                        transpose(t + 1, vcbs[t + 1])
                    if v == 6 and t + 1 < NBT:
                        first_layer_hT(t + 1)
                    if v == 7 and t + 1 < NBT:
                        first_layer_hb(t + 1)
                    if v == 8:
                        target_dot(t)
                    ps = psm.tile([128, VT], F32, tag="ps", name=f"ps{t}_{v}")
                    for lo, w in VCHUNKS:
                        for kg in range(NKG):
                            nc.tensor.matmul(
                                ps[:, lo : lo + w],
                                lhsT=hT8[:, kg, :, ts(t, 128)],
                                rhs=w2sbs[v][:, kg, :, lo : lo + w],
                                start=(kg == 0),
                                stop=(kg == NKG - 1),
                                perf_mode=DR,
                            )
                    if b2_nz:
                        nc.vector.tensor_add(
                            ps[:], ps[:], b2rep_sb[:, ts(v, VT)]
                        )
                    escr = scrp.tile([128, VT], BF16, tag="escr", name=f"e{t}_{v}")
                    nc.scalar.activation(
                        escr[:], ps[:], AF.Exp, scale=1.0 / W2_SCALE
                    )
                    if v == 0:
                        nc.vector.tensor_copy(acc[:], escr[:])
                    else:
                        nc.vector.tensor_add(acc[:], acc[:], escr[:])
                nc.vector.reduce_sum(
                    out=fin[:, t : t + 1], in_=acc[:], axis=mybir.AxisListType.X
                )

            # ---- Phase 3: logsumexp and output.  One batched Ln so the
            # Exp->Ln activation-table swap happens exactly once. ----
            nc.scalar.activation(fin[:, NBT : 2 * NBT], fin[:, :NBT], AF.Ln)
            nc.vector.tensor_sub(
                fin[:, 2 * NBT : 3 * NBT], fin[:, NBT : 2 * NBT], tdot[:, :NBT]
            )
            nc.sync.dma_start(nll[:], fin[:, 2 * NBT : 3 * NBT])

    nc.compile()
    _BUILD_CACHE[key] = nc
    return nc


def _prep_inputs(ws, cs, vectors, W1, b1, W2, b2, vector_to_support):
    ws = np.asarray(ws)
    cs = np.asarray(cs)
    vectors = np.asarray(vectors, dtype=np.float32)
    W1 = np.asarray(W1, dtype=np.float32)
    b1 = np.asarray(b1, dtype=np.float32)
    W2 = np.asarray(W2, dtype=np.float32)
    b2 = np.asarray(b2, dtype=np.float32)
    v2s = np.asarray(vector_to_support)

    b1_nz = bool(np.any(b1))
    b2_nz = bool(np.any(b2))

    w1p = np.zeros((DP, H), dtype=ml_dtypes.bfloat16)
    w1p[:D] = W1.astype(ml_dtypes.bfloat16)
    # fp8 DoubleRow layout, v-chunk-major per partition:
    # w2q[p, v, kg, i, vt] = (W2*S)[kg*256 + i*128 + p, v*VT + vt]
    w2s = (W2 * W2_SCALE).astype(ml_dtypes.float8_e4m3)
    w2q = np.ascontiguousarray(
        w2s.reshape(NKG, 2, 128, NVT, VT).transpose(3, 2, 0, 1, 4)
    )
    w2tb = np.ascontiguousarray(
        np.concatenate([W2.T, b2[:, None]], axis=1).astype(np.float32)
    )
    b1c = np.ascontiguousarray(b1.reshape(NHC, 128).T)

    shared = {
        "vectors": np.ascontiguousarray(vectors),
        "w1": w1p,
        "b1c": b1c,
        "w2q": w2q,
        "w2tb": w2tb,
    }
    if b1_nz:
        shared["b1rep"] = np.ascontiguousarray(
            np.broadcast_to(b1, (128, H)).astype(np.float32)
        )
    if b2_nz:
        shared["b2rep"] = np.ascontiguousarray(
            np.broadcast_to(b2, (128, V)).astype(np.float32)
        )

    # map target word -> support index on the host (pure input indexing)
    ws_sup = v2s[np.asarray(ws)].astype(np.int32)

    in_maps = []
    for c in range(NCORES):
        sl = slice(c * BL, (c + 1) * BL)
        m = dict(shared)
        # host-transposed to [128, NBT]: [p, t] = idx[t*128 + p]
        m["cs_idx"] = np.ascontiguousarray(
            cs[sl].astype(np.int32).reshape(NBT, 128).T
        )
        m["ws_sup"] = np.ascontiguousarray(
            ws_sup[sl].reshape(NBT, 128).T
        )
        in_maps.append(m)
    return in_maps, b1_nz, b2_nz


def run(inputs: dict, trace: bool = False):
    """Run the SPMD kernel. Returns (output [B] fp32, BassKernelResults)."""
    in_maps, b1_nz, b2_nz = _prep_inputs(**inputs)
    nc = _build(b1_nz, b2_nz)
    res = bass_utils.run_bass_kernel_spmd(
        nc, in_maps, core_ids=list(range(NCORES)), trace=trace
    )
    # nll comes back [128, NBT] with [p, t] = row t*128+p
    out = np.concatenate(
        [r["nll"].T.reshape(-1) for r in res.results]
    ).astype(np.float32)
    return out, res


def kernel(**inputs) -> np.ndarray:
    out, _ = run(inputs, trace=False)
    return out


# revision 30
# speedup vs baseline: 1.1540x; 1.1540x over previous
"""Trainium2 Bass kernel for nn_ConditionalSoftmax (sampled-softmax NLL loss).

Computes, for each batch row b:
    v_c   = vectors[cs[b]]                      # [D]
    h     = relu(v_c @ W1 + b1)                 # [H]
    logit = h @ W2 + b2                         # [V]
    nll_b = logsumexp(logit) - logit[v2s[ws[b]]]

Sharding: data-parallel over batch across 8 NeuronCores (1024 rows/core),
weights replicated.  Per core the dominant work is the [1024,512]@[512,20000]
matmul plus the exp of all 20.5M logits.  The matmul runs in fp8_e4m3 with
the PE's DoubleRow perf mode (K=256 per instruction, 2x bf16 throughput);
W2 is pre-scaled by 32 on the host so its values sit in the fp8 normal
range, and the Exp activation's scale parameter undoes the factor for free.
W2 (fp8, 80KB/partition) stays resident in SBUF.  Logits accumulate in
[128,2000] PSUM tiles (4 banks, double buffered) and are reduced in place
by the ScalarEngine's fused exp+row-sum (accum_out), so the [1024,20000]
logit matrix never touches HBM and the per-instruction activation overhead
is amortized over 2000 columns.  The final log runs as ONE batched Ln over
[128,8] so the Exp/Ln activation tables swap exactly once.  The target
logit takes a separate cheap path: indirect-gather of the needed W2.T rows
(fp32) and a multiply-reduce on the VectorEngine against an fp32 recompute
of h.
"""

import numpy as np
import ml_dtypes

import concourse.bass as bass
import concourse.mybir as mybir
import concourse.tile as tile
from concourse import bacc, bass_utils
from concourse.bass import IndirectOffsetOnAxis, ts
from concourse.masks import make_identity

# Problem shapes (hardcoded per contest contract)
N_VOCAB = 50000
V = 20000
D = 300
DP = 384          # D padded to 3*128
NDC = 3           # contraction chunks for D
H = 512
NKG = 2           # DoubleRow contraction groups for H (256 each)
NHC = 4           # 128-row contraction chunks for H
B = 8192
NCORES = 8
BL = B // NCORES  # 1024 rows per core
NBT = BL // 128   # 8 batch tiles of 128 rows
VT = 2000         # vocab tile width (4 PSUM banks)
NVT = V // VT     # 10 vocab tiles
# matmul chunks within a VT tile (cannot cross a 512-f32 PSUM bank boundary)
VCHUNKS = ((0, 512), (512, 512), (1024, 512), (1536, VT - 1536))

W2_SCALE = 32.0   # host pre-scale of W2 into fp8 range; undone by Exp scale

F32 = mybir.dt.float32
BF16 = mybir.dt.bfloat16
FP8 = mybir.dt.float8e4
I32 = mybir.dt.int32
AF = mybir.ActivationFunctionType
OP = mybir.AluOpType
DR = mybir.MatmulPerfMode.DoubleRow

_BUILD_CACHE = {}


def _build(b1_nz: bool, b2_nz: bool):
    key = (b1_nz, b2_nz)
    if key in _BUILD_CACHE:
        return _BUILD_CACHE[key]

    nc = bacc.Bacc(
        "TRN2",
        target_bir_lowering=False,
        debug=False,
        num_devices=NCORES,
        num_swdge_queues=4,
    )

    # Index tensors pre-transposed on the host to [128, NBT] so each loads
    # in ONE cheap DMA (contiguous 32B runs per partition).  ws is already
    # mapped through vector_to_support on the host.
    cs_idx = nc.dram_tensor("cs_idx", [128, NBT], I32, kind="ExternalInput").ap()
    ws_sup = nc.dram_tensor("ws_sup", [128, NBT], I32, kind="ExternalInput").ap()
    vectors = nc.dram_tensor("vectors", [N_VOCAB, D], F32, kind="ExternalInput").ap()
    w1 = nc.dram_tensor("w1", [DP, H], BF16, kind="ExternalInput").ap()
    b1c = nc.dram_tensor("b1c", [128, NHC], F32, kind="ExternalInput").ap()
    # W2 pre-scaled by W2_SCALE, fp8, laid out [v-chunk, p, kg, i, vt] with
    # W2s[kg*256 + i*128 + p, v*VT + vt] so each v-chunk is one fully
    # contiguous 1MB DRAM block (minimal DMA descriptor count — the early
    # window is descriptor-processing-bound across all 16 DMA queues).
    w2q = nc.dram_tensor(
        "w2q", [NVT, 128, NKG, 2, VT], FP8, kind="ExternalInput"
    ).ap()
    w2tb = nc.dram_tensor("w2tb", [V, H + 1], F32, kind="ExternalInput").ap()
    if b1_nz:
        b1rep = nc.dram_tensor("b1rep", [128, H], F32, kind="ExternalInput").ap()
    if b2_nz:
        b2rep = nc.dram_tensor("b2rep", [128, V], F32, kind="ExternalInput").ap()
    nll = nc.dram_tensor("nll", [128, NBT], F32, kind="ExternalOutput").ap()

    with tile.TileContext(nc) as tc:
        with (
            tc.tile_pool(name="consts", bufs=1) as consts,
            tc.tile_pool(name="idx", bufs=8) as idxp,
            tc.tile_pool(name="vc", bufs=8) as vcp,
            tc.tile_pool(name="gw", bufs=4) as gwp,
            tc.tile_pool(name="scr", bufs=2) as scrp,
            tc.tile_pool(name="acc", bufs=2) as accp,
            tc.tile_pool(name="ps", bufs=2, space="PSUM") as psm,
        ):
            # Index DMAs first: they gate the whole phase-1 chain and the
            # Sync sequencer issues DMAs serially (~600ns each).
            cidx = consts.tile([128, NBT], I32)
            nc.sync.dma_start(cidx[:], cs_idx[:])
            widx = consts.tile([128, NBT], I32)
            nc.sync.dma_start(widx[:], ws_sup[:])
            b1sb = consts.tile([128, NHC], F32)
            nc.sync.dma_start(b1sb[:], b1c[:])
            w1sb = consts.tile([128, NDC, H], BF16)
            nc.sync.dma_start(w1sb[:], w1.rearrange("(c p) h -> p c h", p=128))
            if b1_nz:
                b1rep_sb = consts.tile([128, H], F32)
                nc.sync.dma_start(b1rep_sb[:], b1rep[:])
            if b2_nz:
                b2rep_sb = consts.tile([128, V], F32)
                nc.sync.dma_start(b2rep_sb[:], b2rep[:])

            # Resident fp8 W2, loaded in v-chunks so phase 2 can start on
            # chunk 0 while later chunks stream in.  One tile per chunk so
            # dependency tracking is per-chunk, issued on Sync BEFORE the
            # transposes (HWDGE queues are FIFO: these must not sit behind
            # DMAs that wait on the slow gathers).  Keeping them out of the
            # Scalar stream matters even more: the Scalar sequencer is
            # in-order, and DMA flow-control waits there would stall every
            # Exp behind them.
            w2sbs = []
            for v in range(NVT):
                w2sb_v = consts.tile([128, NKG, 2, VT], FP8, name=f"w2sb{v}")
                nc.sync.dma_start(w2sb_v[:], w2q[v])
                w2sbs.append(w2sb_v)

            ident = consts.tile([128, 128], BF16)
            make_identity(nc, ident[:])

            # Long-lived activations
            vcT = consts.tile([128, NDC, BL], BF16)    # v_c^T, d-major
            hT8 = consts.tile([128, NKG, 2, BL], FP8)  # h^T fp8, DoubleRow layout
            hb = consts.tile([128, NBT, H], F32)       # h, batch-major (target dot)
            tdot = consts.tile([128, NBT], F32)        # target logits
            fin = consts.tile([128, 3 * NBT], F32)     # S | lnS | result

            # ---- Phase helpers, software-pipelined into the phase-2 sweep
            # so no engine's in-order stream ever bunches slow work in front
            # of the critical path. ----

            def gather(t):
                """vc embedding gather for batch tile t (gpsimd)."""
                vc = vcp.tile([128, D], F32, tag="vc", name=f"vc{t}")
                nc.gpsimd.indirect_dma_start(
                    out=vc[:],
                    out_offset=None,
                    in_=vectors[:],
                    in_offset=IndirectOffsetOnAxis(ap=cidx[:, t : t + 1], axis=0),
                )
                return vc

            def cast(t, vc):
                """bf16 cast for batch tile t."""
                vcb = vcp.tile([128, DP], BF16, tag="vcb", name=f"vcb{t}")
                nc.vector.memset(vcb[:, D:DP], 0.0)
                nc.vector.tensor_copy(vcb[:, :D], vc[:])
                return vcb

            def transpose(t, vcb):
                """PE transposes for batch tile t.  Kept off the DMA queues
                entirely: the 1MB W2 chunks keep the HWDGE rings busy for
                the first ~40us and any DMA behind them stalls.  Each
                transpose is its own accumulation group, so each goes to its
                own 2KB PSUM bank (1024 bf16 columns)."""
                tpt = psm.tile([128, 2 * VT], BF16, tag="ps", name=f"tp{t}")
                for c in range(NDC):
                    nc.tensor.transpose(
                        tpt[:, c * 1024 : c * 1024 + 128],
                        vcb[:, ts(c, 128)],
                        ident[:],
                    )
                for c in range(NDC):
                    nc.vector.tensor_copy(
                        vcT[:, c, ts(t, 128)], tpt[:, c * 1024 : c * 1024 + 128]
                    )

            def first_layer_hT(t):
                """W1 matmuls + relu + fp8 cast for batch tile t.  Each hc
                chunk is an independent accumulation group, and start=True
                zeroes the whole 2KB PSUM bank, so each group gets its own
                bank (columns hc*512)."""
                pst = psm.tile([128, VT], F32, tag="ps", name=f"pst{t}")
                for hc in range(NHC):
                    for c in range(NDC):
                        nc.tensor.matmul(
                            pst[:, hc * 512 : hc * 512 + 128],
                            lhsT=w1sb[:, c, ts(hc, 128)],
                            rhs=vcT[:, c, ts(t, 128)],
                            start=(c == 0),
                            stop=(c == NDC - 1),
                        )
                # relu + bias, cast to fp8 DoubleRow layout (DVE)
                for hc in range(NHC):
                    nc.vector.tensor_scalar(
                        out=hT8[:, hc // 2, hc % 2, ts(t, 128)],
                        in0=pst[:, hc * 512 : hc * 512 + 128],
                        scalar1=b1sb[:, hc : hc + 1],
                        scalar2=0.0,
                        op0=OP.add,
                        op1=OP.max,
                    )

            def first_layer_hb(t):
                """Batch-major fp32 h for the target-logit dot."""
                psb = psm.tile([128, VT], F32, tag="ps", name=f"psb{t}")
                for c in range(NDC):
                    nc.tensor.matmul(
                        psb[:, :512],
                        lhsT=vcT[:, c, ts(t, 128)],
                        rhs=w1sb[:, c, :],
                        start=(c == 0),
                        stop=(c == NDC - 1),
                    )
                if b1_nz:
                    nc.vector.tensor_add(psb[:, :512], psb[:, :512], b1rep_sb[:])
                nc.vector.tensor_scalar_max(hb[:, t, :], psb[:, :512], 0.0)

            def target_dot(t):
                """Gather W2.T row of the target, fp32 dot with h."""
                g = gwp.tile([128, H + 1], F32, tag="g", name=f"g{t}")
                nc.gpsimd.indirect_dma_start(
                    out=g[:],
                    out_offset=None,
                    in_=w2tb[:],
                    in_offset=IndirectOffsetOnAxis(ap=widx[:, t : t + 1], axis=0),
                )
                # (tensor_tensor_reduce is broken on this HW path; use 3 ops)
                gscr = gwp.tile([128, H], F32, tag="gscr", name=f"gscr{t}")
                nc.vector.tensor_mul(gscr[:], hb[:, t, :], g[:, :H])
                gacc = gwp.tile([128, 1], F32, tag="gacc", name=f"gacc{t}")
                nc.vector.reduce_sum(
                    out=gacc[:], in_=gscr[:], axis=mybir.AxisListType.X
                )
                nc.vector.tensor_add(tdot[:, t : t + 1], gacc[:], g[:, H : H + 1])

            # ---- Main pipeline.  All gathers queue on gpsimd immediately
            # (they pace at several us each due to SWDGE ring flow control,
            # but nothing downstream needs tile t before its sweep).  Per
            # batch-tile sweep over v, the NEXT tile's cast/transpose,
            # first-layer and this tile's target-dot are injected at
            # staggered points so no in-order engine stream ever queues
            # slow work in front of critical work.  The exp row-sums run on
            # the otherwise-idle VectorEngine (bf16, 2x mode) so the Scalar
            # engine does nothing but back-to-back Exp. ----
            vcs = [gather(t) for t in range(NBT)]
            transpose(0, cast(0, vcs[0]))
            first_layer_hT(0)
            first_layer_hb(0)
            vcbs = {}
            for t in range(NBT):
                acc = accp.tile([128, VT], BF16, tag="acc", name=f"acc{t}")
                for v in range(NVT):
                    if v == 2 and t + 1 < NBT:
                        vcbs[t + 1] = cast(t + 1, vcs[t + 1])
                    if v == 4 and t + 1 < NBT:
                        transpose(t + 1, vcbs[t + 1])
                    if v == 6 and t + 1 < NBT:
                        first_layer_hT(t + 1)
                    if v == 7 and t + 1 < NBT:
                        first_layer_hb(t + 1)
                    if v == 8:
                        target_dot(t)
                    ps = psm.tile([128, VT], F32, tag="ps", name=f"ps{t}_{v}")
                    for lo, w in VCHUNKS:
                        for kg in range(NKG):
                            nc.tensor.matmul(
                                ps[:, lo : lo + w],
                                lhsT=hT8[:, kg, :, ts(t, 128)],
                                rhs=w2sbs[v][:, kg, :, lo : lo + w],
                                start=(kg == 0),
                                stop=(kg == NKG - 1),
                                perf_mode=DR,
                            )
                    if b2_nz:
                        nc.vector.tensor_add(
                            ps[:], ps[:], b2rep_sb[:, ts(v, VT)]
                        )
                    escr = scrp.tile([128, VT], BF16, tag="escr", name=f"e{t}_{v}")
                    nc.scalar.activation(
                        escr[:], ps[:], AF.Exp, scale=1.0 / W2_SCALE
                    )
                    if v == 0:
                        nc.vector.tensor_copy(acc[:], escr[:])
                    else:
                        nc.vector.tensor_add(acc[:], acc[:], escr[:])
                nc.vector.reduce_sum(
                    out=fin[:, t : t + 1], in_=acc[:], axis=mybir.AxisListType.X
                )

            # ---- Phase 3: logsumexp and output.  One batched Ln so the
            # Exp->Ln activation-table swap happens exactly once. ----
            nc.scalar.activation(fin[:, NBT : 2 * NBT], fin[:, :NBT], AF.Ln)
            nc.vector.tensor_sub(
                fin[:, 2 * NBT : 3 * NBT], fin[:, NBT : 2 * NBT], tdot[:, :NBT]
            )
            nc.sync.dma_start(nll[:], fin[:, 2 * NBT : 3 * NBT])

    nc.compile()
    _BUILD_CACHE[key] = nc
    return nc


def _prep_inputs(ws, cs, vectors, W1, b1, W2, b2, vector_to_support):
    ws = np.asarray(ws)
    cs = np.asarray(cs)
    vectors = np.asarray(vectors, dtype=np.float32)
    W1 = np.asarray(W1, dtype=np.float32)
    b1 = np.asarray(b1, dtype=np.float32)
    W2 = np.asarray(W2, dtype=np.float32)
    b2 = np.asarray(b2, dtype=np.float32)
    v2s = np.asarray(vector_to_support)

    b1_nz = bool(np.any(b1))
    b2_nz = bool(np.any(b2))

    w1p = np.zeros((DP, H), dtype=ml_dtypes.bfloat16)
    w1p[:D] = W1.astype(ml_dtypes.bfloat16)
    # fp8 DoubleRow layout, v-chunk-major per partition:
    # w2q[p, v, kg, i, vt] = (W2*S)[kg*256 + i*128 + p, v*VT + vt]
    w2s = (W2 * W2_SCALE).astype(ml_dtypes.float8_e4m3)
    w2q = np.ascontiguousarray(
        w2s.reshape(NKG, 2, 128, NVT, VT).transpose(3, 2, 0, 1, 4)
    )
    w2tb = np.ascontiguousarray(
        np.concatenate([W2.T, b2[:, None]], axis=1).astype(np.float32)
    )
    b1c = np.ascontiguousarray(b1.reshape(NHC, 128).T)

    shared = {
        "vectors": np.ascontiguousarray(vectors),
        "w1": w1p,
        "b1c": b1c,
        "w2q": w2q,
        "w2tb": w2tb,
    }
    if b1_nz:
        shared["b1rep"] = np.ascontiguousarray(
            np.broadcast_to(b1, (128, H)).astype(np.float32)
        )
    if b2_nz:
        shared["b2rep"] = np.ascontiguousarray(
            np.broadcast_to(b2, (128, V)).astype(np.float32)
        )

    # map target word -> support index on the host (pure input indexing)
    ws_sup = v2s[np.asarray(ws)].astype(np.int32)

    in_maps = []
    for c in range(NCORES):
        sl = slice(c * BL, (c + 1) * BL)
        m = dict(shared)
        # host-transposed to [128, NBT]: [p, t] = idx[t*128 + p]
        m["cs_idx"] = np.ascontiguousarray(
            cs[sl].astype(np.int32).reshape(NBT, 128).T
        )
        m["ws_sup"] = np.ascontiguousarray(
            ws_sup[sl].reshape(NBT, 128).T
        )
        in_maps.append(m)
    return in_maps, b1_nz, b2_nz


def run(inputs: dict, trace: bool = False):
    """Run the SPMD kernel. Returns (output [B] fp32, BassKernelResults)."""
    in_maps, b1_nz, b2_nz = _prep_inputs(**inputs)
    nc = _build(b1_nz, b2_nz)
    res = bass_utils.run_bass_kernel_spmd(
        nc, in_maps, core_ids=list(range(NCORES)), trace=trace
    )
    # nll comes back [128, NBT] with [p, t] = row t*128+p
    out = np.concatenate(
        [r["nll"].T.reshape(-1) for r in res.results]
    ).astype(np.float32)
    return out, res


def kernel(**inputs) -> np.ndarray:
    out, _ = run(inputs, trace=False)
    return out


# revision 32
# speedup vs baseline: 1.1924x; 1.0333x over previous
"""Trainium2 Bass kernel for nn_ConditionalSoftmax (sampled-softmax NLL loss).

Computes, for each batch row b:
    v_c   = vectors[cs[b]]                      # [D]
    h     = relu(v_c @ W1 + b1)                 # [H]
    logit = h @ W2 + b2                         # [V]
    nll_b = logsumexp(logit) - logit[v2s[ws[b]]]

Sharding: data-parallel over batch across 8 NeuronCores (1024 rows/core),
weights replicated.  Per core the dominant work is the [1024,512]@[512,20000]
matmul plus the exp of all 20.5M logits.  The matmul runs in fp8_e4m3 with
the PE's DoubleRow perf mode (K=256 per instruction, 2x bf16 throughput);
W2 is pre-scaled by 32 on the host so its values sit in the fp8 normal
range, and the Exp activation's scale parameter undoes the factor for free.
W2 (fp8, 80KB/partition) stays resident in SBUF.  Logits accumulate in
[128,2000] PSUM tiles (4 banks, double buffered) and are reduced in place
by the ScalarEngine's fused exp+row-sum (accum_out), so the [1024,20000]
logit matrix never touches HBM and the per-instruction activation overhead
is amortized over 2000 columns.  The final log runs as ONE batched Ln over
[128,8] so the Exp/Ln activation tables swap exactly once.  The target
logit takes a separate cheap path: indirect-gather of the needed W2.T rows
(fp32) and a multiply-reduce on the VectorEngine against an fp32 recompute
of h.
"""

import numpy as np
import ml_dtypes

import concourse.bass as bass
import concourse.mybir as mybir
import concourse.tile as tile
from concourse import bacc, bass_utils
from concourse.bass import IndirectOffsetOnAxis, ts
from concourse.masks import make_identity

# Problem shapes (hardcoded per contest contract)
N_VOCAB = 50000
V = 20000
D = 300
DP = 384          # D padded to 3*128
NDC = 3           # contraction chunks for D
H = 512
NKG = 2           # DoubleRow contraction groups for H (256 each)
NHC = 4           # 128-row contraction chunks for H
B = 8192
NCORES = 8
BL = B // NCORES  # 1024 rows per core
NBT = BL // 128   # 8 batch tiles of 128 rows
VT = 2000         # vocab tile width (4 PSUM banks)
NVT = V // VT     # 10 vocab tiles
# matmul chunks within a VT tile (cannot cross a 512-f32 PSUM bank boundary)
VCHUNKS = ((0, 512), (512, 512), (1024, 512), (1536, VT - 1536))

W2_SCALE = 32.0   # host pre-scale of W2 into fp8 range; undone by Exp scale

F32 = mybir.dt.float32
BF16 = mybir.dt.bfloat16
FP8 = mybir.dt.float8e4
I32 = mybir.dt.int32
AF = mybir.ActivationFunctionType
OP = mybir.AluOpType
DR = mybir.MatmulPerfMode.DoubleRow

_BUILD_CACHE = {}


def _build(b1_nz: bool, b2_nz: bool):
    key = (b1_nz, b2_nz)
    if key in _BUILD_CACHE:
        return _BUILD_CACHE[key]

    nc = bacc.Bacc(
        "TRN2",
        target_bir_lowering=False,
        debug=False,
        num_devices=NCORES,
        num_swdge_queues=2,
    )

    # Index tensors pre-transposed on the host to [128, NBT] so each loads
    # in ONE cheap DMA (contiguous 32B runs per partition).  ws is already
    # mapped through vector_to_support on the host.
    cs_idx = nc.dram_tensor("cs_idx", [128, NBT], I32, kind="ExternalInput").ap()
    ws_sup = nc.dram_tensor("ws_sup", [128, NBT], I32, kind="ExternalInput").ap()
    vectors = nc.dram_tensor("vectors", [N_VOCAB, D], F32, kind="ExternalInput").ap()
    w1 = nc.dram_tensor("w1", [DP, H], BF16, kind="ExternalInput").ap()
    b1c = nc.dram_tensor("b1c", [128, NHC], F32, kind="ExternalInput").ap()
    # W2 pre-scaled by W2_SCALE, fp8, laid out [v-chunk, p, kg, i, vt] with
    # W2s[kg*256 + i*128 + p, v*VT + vt] so each v-chunk is one fully
    # contiguous 1MB DRAM block (minimal DMA descriptor count — the early
    # window is descriptor-processing-bound across all 16 DMA queues).
    w2q = nc.dram_tensor(
        "w2q", [NVT, 128, NKG, 2, VT], FP8, kind="ExternalInput"
    ).ap()
    w2tb = nc.dram_tensor("w2tb", [V, H + 1], F32, kind="ExternalInput").ap()
    if b1_nz:
        b1rep = nc.dram_tensor("b1rep", [128, H], F32, kind="ExternalInput").ap()
    if b2_nz:
        b2rep = nc.dram_tensor("b2rep", [128, V], F32, kind="ExternalInput").ap()
    nll = nc.dram_tensor("nll", [128, NBT], F32, kind="ExternalOutput").ap()

    with tile.TileContext(nc) as tc:
        with (
            tc.tile_pool(name="consts", bufs=1) as consts,
            tc.tile_pool(name="idx", bufs=8) as idxp,
            tc.tile_pool(name="vc", bufs=8) as vcp,
            tc.tile_pool(name="gw", bufs=4) as gwp,
            tc.tile_pool(name="scr", bufs=3) as scrp,
            tc.tile_pool(name="acc", bufs=2) as accp,
            tc.tile_pool(name="ps", bufs=2, space="PSUM") as psm,
        ):
            # Early DMA issue order is the head latency: the Sync sequencer
            # issues serially (~600ns each) and the queues drain in FIFO
            # order.  The first phase-2 tile needs cidx (gather chain) and
            # W2 chunk 0, so those go first.
            cidx = consts.tile([128, NBT], I32)
            nc.sync.dma_start(cidx[:], cs_idx[:])
            widx = consts.tile([128, NBT], I32)
            nc.sync.dma_start(widx[:], ws_sup[:])

            # Resident fp8 W2, one tile per v-chunk so dependency tracking
            # is per-chunk and phase 2 starts as soon as chunk 0 lands.
            # Issued on Sync (NOT Scalar: the Scalar sequencer is in-order
            # and DMA flow-control waits there would stall every Exp).
            w2sbs = []
            for v in range(NVT):
                w2sbs.append(
                    consts.tile([128, NKG, 2, VT], FP8, name=f"w2sb{v}")
                )

            def load_w2(v):
                nc.sync.dma_start(w2sbs[v][:], w2q[v])

            load_w2(0)
            load_w2(1)
            b1sb = consts.tile([128, NHC], F32)
            nc.sync.dma_start(b1sb[:], b1c[:])
            w1sb = consts.tile([128, NDC, H], BF16)
            nc.sync.dma_start(w1sb[:], w1.rearrange("(c p) h -> p c h", p=128))
            for v in range(2, NVT):
                load_w2(v)
            if b1_nz:
                b1rep_sb = consts.tile([128, H], F32)
                nc.sync.dma_start(b1rep_sb[:], b1rep[:])
            if b2_nz:
                b2rep_sb = consts.tile([128, V], F32)
                nc.sync.dma_start(b2rep_sb[:], b2rep[:])

            ident = consts.tile([128, 128], BF16)
            make_identity(nc, ident[:])

            # Long-lived activations
            vcT = consts.tile([128, NDC, BL], BF16)    # v_c^T, d-major
            hT8 = consts.tile([128, NKG, 2, BL], FP8)  # h^T fp8, DoubleRow layout
            hb = consts.tile([128, NBT, H], F32)       # h, batch-major (target dot)
            tdot = consts.tile([128, NBT], F32)        # target logits
            fin = consts.tile([128, 3 * NBT], F32)     # S | lnS | result

            # ---- Phase helpers, software-pipelined into the phase-2 sweep
            # so no engine's in-order stream ever bunches slow work in front
            # of the critical path. ----

            def gather(t):
                """vc embedding gather for batch tile t (gpsimd)."""
                vc = vcp.tile([128, D], F32, tag="vc", name=f"vc{t}")
                nc.gpsimd.indirect_dma_start(
                    out=vc[:],
                    out_offset=None,
                    in_=vectors[:],
                    in_offset=IndirectOffsetOnAxis(ap=cidx[:, t : t + 1], axis=0),
                )
                return vc

            def cast(t, vc):
                """bf16 cast for batch tile t."""
                vcb = vcp.tile([128, DP], BF16, tag="vcb", name=f"vcb{t}")
                nc.vector.memset(vcb[:, D:DP], 0.0)
                nc.vector.tensor_copy(vcb[:, :D], vc[:])
                return vcb

            def transpose(t, vcb):
                """PE transposes for batch tile t.  Kept off the DMA queues
                entirely: the 1MB W2 chunks keep the HWDGE rings busy for
                the first ~40us and any DMA behind them stalls.  Each
                transpose is its own accumulation group, so each goes to its
                own 2KB PSUM bank (1024 bf16 columns)."""
                tpt = psm.tile([128, 2 * VT], BF16, tag="ps", name=f"tp{t}")
                for c in range(NDC):
                    nc.tensor.transpose(
                        tpt[:, c * 1024 : c * 1024 + 128],
                        vcb[:, ts(c, 128)],
                        ident[:],
                    )
                for c in range(NDC):
                    nc.vector.tensor_copy(
                        vcT[:, c, ts(t, 128)], tpt[:, c * 1024 : c * 1024 + 128]
                    )

            def first_layer_hT(t):
                """W1 matmuls + relu + fp8 cast for batch tile t.  Each hc
                chunk is an independent accumulation group, and start=True
                zeroes the whole 2KB PSUM bank, so each group gets its own
                bank (columns hc*512)."""
                pst = psm.tile([128, VT], F32, tag="ps", name=f"pst{t}")
                for hc in range(NHC):
                    for c in range(NDC):
                        nc.tensor.matmul(
                            pst[:, hc * 512 : hc * 512 + 128],
                            lhsT=w1sb[:, c, ts(hc, 128)],
                            rhs=vcT[:, c, ts(t, 128)],
                            start=(c == 0),
                            stop=(c == NDC - 1),
                        )
                # relu + bias, cast to fp8 DoubleRow layout (DVE)
                for hc in range(NHC):
                    nc.vector.tensor_scalar(
                        out=hT8[:, hc // 2, hc % 2, ts(t, 128)],
                        in0=pst[:, hc * 512 : hc * 512 + 128],
                        scalar1=b1sb[:, hc : hc + 1],
                        scalar2=0.0,
                        op0=OP.add,
                        op1=OP.max,
                    )

            def first_layer_hb(t):
                """Batch-major fp32 h for the target-logit dot."""
                psb = psm.tile([128, VT], F32, tag="ps", name=f"psb{t}")
                for c in range(NDC):
                    nc.tensor.matmul(
                        psb[:, :512],
                        lhsT=vcT[:, c, ts(t, 128)],
                        rhs=w1sb[:, c, :],
                        start=(c == 0),
                        stop=(c == NDC - 1),
                    )
                if b1_nz:
                    nc.vector.tensor_add(psb[:, :512], psb[:, :512], b1rep_sb[:])
                nc.vector.tensor_scalar_max(hb[:, t, :], psb[:, :512], 0.0)

            def target_dot(t):
                """Gather W2.T row of the target, fp32 dot with h."""
                g = gwp.tile([128, H + 1], F32, tag="g", name=f"g{t}")
                nc.gpsimd.indirect_dma_start(
                    out=g[:],
                    out_offset=None,
                    in_=w2tb[:],
                    in_offset=IndirectOffsetOnAxis(ap=widx[:, t : t + 1], axis=0),
                )
                # (tensor_tensor_reduce is broken on this HW path; use 3 ops)
                gscr = gwp.tile([128, H], F32, tag="gscr", name=f"gscr{t}")
                nc.vector.tensor_mul(gscr[:], hb[:, t, :], g[:, :H])
                gacc = gwp.tile([128, 1], F32, tag="gacc", name=f"gacc{t}")
                nc.vector.reduce_sum(
                    out=gacc[:], in_=gscr[:], axis=mybir.AxisListType.X
                )
                nc.vector.tensor_add(tdot[:, t : t + 1], gacc[:], g[:, H : H + 1])

            # ---- Main pipeline.  All gathers queue on gpsimd immediately
            # (they pace at several us each due to SWDGE ring flow control,
            # but nothing downstream needs tile t before its sweep).  Per
            # batch-tile sweep over v, the NEXT tile's cast/transpose,
            # first-layer and this tile's target-dot are injected at
            # staggered points so no in-order engine stream ever queues
            # slow work in front of critical work.  The exp row-sums run on
            # the otherwise-idle VectorEngine (bf16, 2x mode) so the Scalar
            # engine does nothing but back-to-back Exp. ----
            vcs = [gather(t) for t in range(NBT)]
            transpose(0, cast(0, vcs[0]))
            first_layer_hT(0)
            first_layer_hb(0)
            vcbs = {}
            for t in range(NBT):
                acc = accp.tile([128, VT], BF16, tag="acc", name=f"acc{t}")
                for v in range(NVT):
                    if v == 2 and t + 1 < NBT:
                        vcbs[t + 1] = cast(t + 1, vcs[t + 1])
                    if v == 4 and t + 1 < NBT:
                        transpose(t + 1, vcbs[t + 1])
                    if v == 6 and t + 1 < NBT:
                        first_layer_hT(t + 1)
                    if v == 7 and t + 1 < NBT:
                        first_layer_hb(t + 1)
                    if v == 8:
                        target_dot(t)
                    ps = psm.tile([128, VT], F32, tag="ps", name=f"ps{t}_{v}")
                    for lo, w in VCHUNKS:
                        for kg in range(NKG):
                            nc.tensor.matmul(
                                ps[:, lo : lo + w],
                                lhsT=hT8[:, kg, :, ts(t, 128)],
                                rhs=w2sbs[v][:, kg, :, lo : lo + w],
                                start=(kg == 0),
                                stop=(kg == NKG - 1),
                                perf_mode=DR,
                            )
                    if b2_nz:
                        nc.vector.tensor_add(
                            ps[:], ps[:], b2rep_sb[:, ts(v, VT)]
                        )
                    escr = scrp.tile([128, VT], BF16, tag="escr", name=f"e{t}_{v}")
                    nc.scalar.activation(
                        escr[:], ps[:], AF.Exp, scale=1.0 / W2_SCALE
                    )
                    if v == 0:
                        nc.vector.tensor_copy(acc[:], escr[:])
                    else:
                        nc.vector.tensor_add(acc[:], acc[:], escr[:])
                nc.vector.reduce_sum(
                    out=fin[:, t : t + 1], in_=acc[:], axis=mybir.AxisListType.X
                )

            # ---- Phase 3: logsumexp and output.  One batched Ln so the
            # Exp->Ln activation-table swap happens exactly once. ----
            nc.scalar.activation(fin[:, NBT : 2 * NBT], fin[:, :NBT], AF.Ln)
            nc.vector.tensor_sub(
                fin[:, 2 * NBT : 3 * NBT], fin[:, NBT : 2 * NBT], tdot[:, :NBT]
            )
            nc.sync.dma_start(nll[:], fin[:, 2 * NBT : 3 * NBT])

    nc.compile()
    _BUILD_CACHE[key] = nc
    return nc


def _prep_inputs(ws, cs, vectors, W1, b1, W2, b2, vector_to_support):
    ws = np.asarray(ws)
    cs = np.asarray(cs)
    vectors = np.asarray(vectors, dtype=np.float32)
    W1 = np.asarray(W1, dtype=np.float32)
    b1 = np.asarray(b1, dtype=np.float32)
    W2 = np.asarray(W2, dtype=np.float32)
    b2 = np.asarray(b2, dtype=np.float32)
    v2s = np.asarray(vector_to_support)

    b1_nz = bool(np.any(b1))
    b2_nz = bool(np.any(b2))

    w1p = np.zeros((DP, H), dtype=ml_dtypes.bfloat16)
    w1p[:D] = W1.astype(ml_dtypes.bfloat16)
    # fp8 DoubleRow layout, v-chunk-major per partition:
    # w2q[p, v, kg, i, vt] = (W2*S)[kg*256 + i*128 + p, v*VT + vt]
    w2s = (W2 * W2_SCALE).astype(ml_dtypes.float8_e4m3)
    w2q = np.ascontiguousarray(
        w2s.reshape(NKG, 2, 128, NVT, VT).transpose(3, 2, 0, 1, 4)
    )
    w2tb = np.ascontiguousarray(
        np.concatenate([W2.T, b2[:, None]], axis=1).astype(np.float32)
    )
    b1c = np.ascontiguousarray(b1.reshape(NHC, 128).T)

    shared = {
        "vectors": np.ascontiguousarray(vectors),
        "w1": w1p,
        "b1c": b1c,
        "w2q": w2q,
        "w2tb": w2tb,
    }
    if b1_nz:
        shared["b1rep"] = np.ascontiguousarray(
            np.broadcast_to(b1, (128, H)).astype(np.float32)
        )
    if b2_nz:
        shared["b2rep"] = np.ascontiguousarray(
            np.broadcast_to(b2, (128, V)).astype(np.float32)
        )

    # map target word -> support index on the host (pure input indexing)
    ws_sup = v2s[np.asarray(ws)].astype(np.int32)

    in_maps = []
    for c in range(NCORES):
        sl = slice(c * BL, (c + 1) * BL)
        m = dict(shared)
        # host-transposed to [128, NBT]: [p, t] = idx[t*128 + p]
        m["cs_idx"] = np.ascontiguousarray(
            cs[sl].astype(np.int32).reshape(NBT, 128).T
        )
        m["ws_sup"] = np.ascontiguousarray(
            ws_sup[sl].reshape(NBT, 128).T
        )
        in_maps.append(m)
    return in_maps, b1_nz, b2_nz


def run(inputs: dict, trace: bool = False):
    """Run the SPMD kernel. Returns (output [B] fp32, BassKernelResults)."""
    in_maps, b1_nz, b2_nz = _prep_inputs(**inputs)
    nc = _build(b1_nz, b2_nz)
    res = bass_utils.run_bass_kernel_spmd(
        nc, in_maps, core_ids=list(range(NCORES)), trace=trace
    )
    # nll comes back [128, NBT] with [p, t] = row t*128+p
    out = np.concatenate(
        [r["nll"].T.reshape(-1) for r in res.results]
    ).astype(np.float32)
    return out, res


def kernel(**inputs) -> np.ndarray:
    out, _ = run(inputs, trace=False)
    return out


# revision 36
# speedup vs baseline: 1.3688x; 1.1479x over previous
"""Trainium2 Bass kernel for nn_ConditionalSoftmax (sampled-softmax NLL loss).

Computes, for each batch row b:
    v_c   = vectors[cs[b]]                      # [D]
    h     = relu(v_c @ W1 + b1)                 # [H]
    logit = h @ W2 + b2                         # [V]
    nll_b = logsumexp(logit) - logit[v2s[ws[b]]]

Sharding: data-parallel over batch across 8 NeuronCores (1024 rows/core),
weights replicated.  Per core the dominant work is the [1024,512]@[512,20000]
matmul plus the exp of all 20.5M logits.  The matmul runs in fp8_e4m3 with
the PE's DoubleRow perf mode (K=256 per instruction, 2x bf16 throughput);
W2 is pre-scaled by 32 on the host so its values sit in the fp8 normal
range, and the Exp activation's scale parameter undoes the factor for free.
W2 (fp8, 80KB/partition) stays resident in SBUF.  Logits accumulate in
[128,2000] PSUM tiles (4 banks, double buffered) and are reduced in place
by the ScalarEngine's fused exp+row-sum (accum_out), so the [1024,20000]
logit matrix never touches HBM and the per-instruction activation overhead
is amortized over 2000 columns.  The final log runs as ONE batched Ln over
[128,8] so the Exp/Ln activation tables swap exactly once.  The target
logit takes a separate cheap path: indirect-gather of the needed W2.T rows
(fp32) and a multiply-reduce on the VectorEngine against an fp32 recompute
of h.
"""

import numpy as np
import ml_dtypes

import concourse.bass as bass
import concourse.mybir as mybir
import concourse.tile as tile
from concourse import bacc, bass_utils
from concourse.bass import IndirectOffsetOnAxis, ts
from concourse.masks import make_identity

# Problem shapes (hardcoded per contest contract)
N_VOCAB = 50000
V = 20000
D = 300
DP = 384          # D padded to 3*128
NDC = 3           # contraction chunks for D
H = 512
NKG = 2           # DoubleRow contraction groups for H (256 each)
NHC = 4           # 128-row contraction chunks for H
B = 8192
NCORES = 8
BL = B // NCORES  # 1024 rows per core
NBT = BL // 128   # 8 batch tiles of 128 rows
VT = 1000         # vocab tile width (2 PSUM banks)
NVT = V // VT     # 20 vocab tiles
# matmul chunks within a VT tile (cannot cross a 512-f32 PSUM bank boundary)
VCHUNKS = ((0, 512), (512, VT - 512))

W2_SCALE = 32.0   # host pre-scale of W2 into fp8 range; undone by Exp scale

F32 = mybir.dt.float32
BF16 = mybir.dt.bfloat16
FP8 = mybir.dt.float8e4
I32 = mybir.dt.int32
AF = mybir.ActivationFunctionType
OP = mybir.AluOpType
DR = mybir.MatmulPerfMode.DoubleRow

_BUILD_CACHE = {}


def _build(b1_nz: bool, b2_nz: bool):
    key = (b1_nz, b2_nz)
    if key in _BUILD_CACHE:
        return _BUILD_CACHE[key]

    nc = bacc.Bacc(
        "TRN2",
        target_bir_lowering=False,
        debug=False,
        num_devices=NCORES,
        num_swdge_queues=2,
    )

    # Index tensors pre-transposed on the host to [128, NBT] so each loads
    # in ONE cheap DMA (contiguous 32B runs per partition).  ws is already
    # mapped through vector_to_support on the host.
    cs_idx = nc.dram_tensor("cs_idx", [128, NBT], I32, kind="ExternalInput").ap()
    ws_sup = nc.dram_tensor("ws_sup", [128, NBT], I32, kind="ExternalInput").ap()
    vectors = nc.dram_tensor("vectors", [N_VOCAB, D], F32, kind="ExternalInput").ap()
    w1 = nc.dram_tensor("w1", [DP, H], BF16, kind="ExternalInput").ap()
    b1c = nc.dram_tensor("b1c", [128, NHC], F32, kind="ExternalInput").ap()
    # W2 pre-scaled by W2_SCALE, fp8, laid out [v-chunk, p, kg, i, vt] with
    # W2s[kg*256 + i*128 + p, v*VT + vt] so each v-chunk is one fully
    # contiguous 1MB DRAM block (minimal DMA descriptor count — the early
    # window is descriptor-processing-bound across all 16 DMA queues).
    w2q = nc.dram_tensor(
        "w2q", [NVT, 128, NKG, 2, VT], FP8, kind="ExternalInput"
    ).ap()
    w2tb = nc.dram_tensor("w2tb", [V, H + 1], F32, kind="ExternalInput").ap()
    if b1_nz:
        b1rep = nc.dram_tensor("b1rep", [128, H], F32, kind="ExternalInput").ap()
    if b2_nz:
        b2rep = nc.dram_tensor("b2rep", [128, V], F32, kind="ExternalInput").ap()
    nll = nc.dram_tensor("nll", [128, NBT], F32, kind="ExternalOutput").ap()

    with tile.TileContext(nc) as tc:
        with (
            tc.tile_pool(name="consts", bufs=1) as consts,
            tc.tile_pool(name="idx", bufs=8) as idxp,
            tc.tile_pool(name="vc", bufs=8) as vcp,
            tc.tile_pool(name="gw", bufs=4) as gwp,
            tc.tile_pool(name="scr", bufs=3) as scrp,
            tc.tile_pool(name="acc", bufs=2) as accp,
            tc.tile_pool(name="ps", bufs=3, space="PSUM") as psm,
        ):
            # Early DMA issue order is the head latency: the Sync sequencer
            # issues serially (~600ns each) and the queues drain in FIFO
            # order.  The first phase-2 tile needs cidx (gather chain) and
            # W2 chunk 0, so those go first.
            cidx = consts.tile([128, NBT], I32)
            nc.sync.dma_start(cidx[:], cs_idx[:])
            widx = consts.tile([128, NBT], I32)
            nc.sync.dma_start(widx[:], ws_sup[:])

            # Resident fp8 W2, one tile per v-chunk so dependency tracking
            # is per-chunk and phase 2 starts as soon as chunk 0 lands.
            # Issued on Sync (NOT Scalar: the Scalar sequencer is in-order
            # and DMA flow-control waits there would stall every Exp).
            w2sbs = []
            for v in range(NVT):
                w2sbs.append(
                    consts.tile([128, NKG, 2, VT], FP8, name=f"w2sb{v}")
                )

            def load_w2(v):
                nc.sync.dma_start(w2sbs[v][:], w2q[v])

            load_w2(0)
            load_w2(1)
            load_w2(2)
            load_w2(3)
            b1sb = consts.tile([128, NHC], F32)
            nc.sync.dma_start(b1sb[:], b1c[:])
            w1sb = consts.tile([128, NDC, H], BF16)
            nc.sync.dma_start(w1sb[:], w1.rearrange("(c p) h -> p c h", p=128))
            for v in range(4, NVT):
                load_w2(v)
            if b1_nz:
                b1rep_sb = consts.tile([128, H], F32)
                nc.sync.dma_start(b1rep_sb[:], b1rep[:])
            if b2_nz:
                b2rep_sb = consts.tile([128, V], F32)
                nc.sync.dma_start(b2rep_sb[:], b2rep[:])

            ident = consts.tile([128, 128], BF16)
            make_identity(nc, ident[:])

            # Long-lived activations
            vcT = consts.tile([128, NDC, BL], BF16)    # v_c^T, d-major
            hT8 = consts.tile([128, NKG, 2, BL], FP8)  # h^T fp8, DoubleRow layout
            hb = consts.tile([128, NBT, H], F32)       # h, batch-major (target dot)
            tdot = consts.tile([128, NBT], F32)        # target logits
            fin = consts.tile([128, 3 * NBT], F32)     # S | lnS | result

            # ---- Phase helpers, software-pipelined into the phase-2 sweep
            # so no engine's in-order stream ever bunches slow work in front
            # of the critical path. ----

            def gather(t):
                """vc embedding gather for batch tile t (gpsimd)."""
                vc = vcp.tile([128, D], F32, tag="vc", name=f"vc{t}")
                nc.gpsimd.indirect_dma_start(
                    out=vc[:],
                    out_offset=None,
                    in_=vectors[:],
                    in_offset=IndirectOffsetOnAxis(ap=cidx[:, t : t + 1], axis=0),
                )
                return vc

            def cast(t, vc):
                """bf16 cast for batch tile t."""
                vcb = vcp.tile([128, DP], BF16, tag="vcb", name=f"vcb{t}")
                nc.vector.memset(vcb[:, D:DP], 0.0)
                nc.vector.tensor_copy(vcb[:, :D], vc[:])
                return vcb

            def transpose(t, vcb):
                """PE transposes for batch tile t.  Kept off the DMA queues
                entirely: the W2 chunks keep the HWDGE rings busy for the
                first ~40us and any DMA behind them stalls.  Each transpose
                is its own accumulation group, so each goes to its own 2KB
                PSUM bank (1024 bf16 columns; a [128,2*VT] bf16 tile holds
                two banks, so the three d-chunks use two tiles)."""
                tp1 = psm.tile([128, 2 * VT], BF16, tag="ps", name=f"tp1_{t}")
                tp2 = psm.tile([128, 2 * VT], BF16, tag="ps", name=f"tp2_{t}")
                outs = [tp1[:, 0:128], tp1[:, 1024:1152], tp2[:, 0:128]]
                for c in range(NDC):
                    nc.tensor.transpose(outs[c], vcb[:, ts(c, 128)], ident[:])
                for c in range(NDC):
                    nc.vector.tensor_copy(vcT[:, c, ts(t, 128)], outs[c])

            def first_layer_hT(t, half):
                """W1 matmuls + relu + fp8 cast for batch tile t, two hc
                chunks per call.  Each hc chunk is an independent
                accumulation group, and start=True zeroes the whole 2KB
                PSUM bank, so each group gets its own bank."""
                pst = psm.tile([128, VT], F32, tag="ps", name=f"pst{t}_{half}")
                for i in range(2):
                    hc = 2 * half + i
                    for c in range(NDC):
                        nc.tensor.matmul(
                            pst[:, i * 512 : i * 512 + 128],
                            lhsT=w1sb[:, c, ts(hc, 128)],
                            rhs=vcT[:, c, ts(t, 128)],
                            start=(c == 0),
                            stop=(c == NDC - 1),
                        )
                # relu + bias, cast to fp8 DoubleRow layout (DVE)
                for i in range(2):
                    hc = 2 * half + i
                    nc.vector.tensor_scalar(
                        out=hT8[:, hc // 2, hc % 2, ts(t, 128)],
                        in0=pst[:, i * 512 : i * 512 + 128],
                        scalar1=b1sb[:, hc : hc + 1],
                        scalar2=0.0,
                        op0=OP.add,
                        op1=OP.max,
                    )

            def first_layer_hb(t):
                """Batch-major fp32 h for the target-logit dot."""
                psb = psm.tile([128, VT], F32, tag="ps", name=f"psb{t}")
                for c in range(NDC):
                    nc.tensor.matmul(
                        psb[:, :512],
                        lhsT=vcT[:, c, ts(t, 128)],
                        rhs=w1sb[:, c, :],
                        start=(c == 0),
                        stop=(c == NDC - 1),
                    )
                if b1_nz:
                    nc.vector.tensor_add(psb[:, :512], psb[:, :512], b1rep_sb[:])
                nc.vector.tensor_scalar_max(hb[:, t, :], psb[:, :512], 0.0)

            def target_dot(t):
                """Gather W2.T row of the target, fp32 dot with h."""
                g = gwp.tile([128, H + 1], F32, tag="g", name=f"g{t}")
                nc.gpsimd.indirect_dma_start(
                    out=g[:],
                    out_offset=None,
                    in_=w2tb[:],
                    in_offset=IndirectOffsetOnAxis(ap=widx[:, t : t + 1], axis=0),
                )
                # (tensor_tensor_reduce is broken on this HW path; use 3 ops)
                gscr = gwp.tile([128, H], F32, tag="gscr", name=f"gscr{t}")
                nc.vector.tensor_mul(gscr[:], hb[:, t, :], g[:, :H])
                gacc = gwp.tile([128, 1], F32, tag="gacc", name=f"gacc{t}")
                nc.vector.reduce_sum(
                    out=gacc[:], in_=gscr[:], axis=mybir.AxisListType.X
                )
                nc.vector.tensor_add(tdot[:, t : t + 1], gacc[:], g[:, H : H + 1])

            # ---- Main pipeline.  All gathers queue on gpsimd immediately
            # (they pace at several us each due to SWDGE ring flow control,
            # but nothing downstream needs tile t before its sweep).  Per
            # batch-tile sweep over v, the NEXT tile's cast/transpose,
            # first-layer and this tile's target-dot are injected at
            # staggered points so no in-order engine stream ever queues
            # slow work in front of critical work.  The exp row-sums run on
            # the otherwise-idle VectorEngine (bf16, 2x mode) so the Scalar
            # engine does nothing but back-to-back Exp. ----
            vcs = [gather(t) for t in range(NBT)]
            transpose(0, cast(0, vcs[0]))
            first_layer_hT(0, 0)
            first_layer_hT(0, 1)
            first_layer_hb(0)
            vcbs = {}
            for t in range(NBT):
                acc = accp.tile([128, VT], BF16, tag="acc", name=f"acc{t}")
                for v in range(NVT):
                    if v == 4 and t + 1 < NBT:
                        vcbs[t + 1] = cast(t + 1, vcs[t + 1])
                    if v == 8 and t + 1 < NBT:
                        transpose(t + 1, vcbs[t + 1])
                    if v == 12 and t + 1 < NBT:
                        first_layer_hT(t + 1, 0)
                    if v == 13 and t + 1 < NBT:
                        first_layer_hT(t + 1, 1)
                    if v == 15 and t + 1 < NBT:
                        first_layer_hb(t + 1)
                    if v == 17:
                        target_dot(t)
                    ps = psm.tile([128, VT], F32, tag="ps", name=f"ps{t}_{v}")
                    for lo, w in VCHUNKS:
                        for kg in range(NKG):
                            nc.tensor.matmul(
                                ps[:, lo : lo + w],
                                lhsT=hT8[:, kg, :, ts(t, 128)],
                                rhs=w2sbs[v][:, kg, :, lo : lo + w],
                                start=(kg == 0),
                                stop=(kg == NKG - 1),
                                perf_mode=DR,
                            )
                    if b2_nz:
                        nc.vector.tensor_add(
                            ps[:], ps[:], b2rep_sb[:, ts(v, VT)]
                        )
                    escr = scrp.tile([128, VT], BF16, tag="escr", name=f"e{t}_{v}")
                    nc.scalar.activation(
                        escr[:], ps[:], AF.Exp, scale=1.0 / W2_SCALE
                    )
                    if v == 0:
                        nc.vector.tensor_copy(acc[:], escr[:])
                    else:
                        nc.vector.tensor_add(acc[:], acc[:], escr[:])
                nc.vector.reduce_sum(
                    out=fin[:, t : t + 1], in_=acc[:], axis=mybir.AxisListType.X
                )

            # ---- Phase 3: logsumexp and output.  One batched Ln so the
            # Exp->Ln activation-table swap happens exactly once. ----
            nc.scalar.activation(fin[:, NBT : 2 * NBT], fin[:, :NBT], AF.Ln)
            nc.vector.tensor_sub(
                fin[:, 2 * NBT : 3 * NBT], fin[:, NBT : 2 * NBT], tdot[:, :NBT]
            )
            nc.sync.dma_start(nll[:], fin[:, 2 * NBT : 3 * NBT])

    nc.compile()
    _BUILD_CACHE[key] = nc
    return nc


def _prep_inputs(ws, cs, vectors, W1, b1, W2, b2, vector_to_support):
    ws = np.asarray(ws)
    cs = np.asarray(cs)
    vectors = np.asarray(vectors, dtype=np.float32)
    W1 = np.asarray(W1, dtype=np.float32)
    b1 = np.asarray(b1, dtype=np.float32)
    W2 = np.asarray(W2, dtype=np.float32)
    b2 = np.asarray(b2, dtype=np.float32)
    v2s = np.asarray(vector_to_support)

    b1_nz = bool(np.any(b1))
    b2_nz = bool(np.any(b2))

    w1p = np.zeros((DP, H), dtype=ml_dtypes.bfloat16)
    w1p[:D] = W1.astype(ml_dtypes.bfloat16)
    # fp8 DoubleRow layout, v-chunk-major per partition:
    # w2q[p, v, kg, i, vt] = (W2*S)[kg*256 + i*128 + p, v*VT + vt]
    w2s = (W2 * W2_SCALE).astype(ml_dtypes.float8_e4m3)
    w2q = np.ascontiguousarray(
        w2s.reshape(NKG, 2, 128, NVT, VT).transpose(3, 2, 0, 1, 4)
    )
    w2tb = np.ascontiguousarray(
        np.concatenate([W2.T, b2[:, None]], axis=1).astype(np.float32)
    )
    b1c = np.ascontiguousarray(b1.reshape(NHC, 128).T)

    shared = {
        "vectors": np.ascontiguousarray(vectors),
        "w1": w1p,
        "b1c": b1c,
        "w2q": w2q,
        "w2tb": w2tb,
    }
    if b1_nz:
        shared["b1rep"] = np.ascontiguousarray(
            np.broadcast_to(b1, (128, H)).astype(np.float32)
        )
    if b2_nz:
        shared["b2rep"] = np.ascontiguousarray(
            np.broadcast_to(b2, (128, V)).astype(np.float32)
        )

    # map target word -> support index on the host (pure input indexing)
    ws_sup = v2s[np.asarray(ws)].astype(np.int32)

    in_maps = []
    for c in range(NCORES):
        sl = slice(c * BL, (c + 1) * BL)
        m = dict(shared)
        # host-transposed to [128, NBT]: [p, t] = idx[t*128 + p]
        m["cs_idx"] = np.ascontiguousarray(
            cs[sl].astype(np.int32).reshape(NBT, 128).T
        )
        m["ws_sup"] = np.ascontiguousarray(
            ws_sup[sl].reshape(NBT, 128).T
        )
        in_maps.append(m)
    return in_maps, b1_nz, b2_nz


def run(inputs: dict, trace: bool = False):
    """Run the SPMD kernel. Returns (output [B] fp32, BassKernelResults)."""
    in_maps, b1_nz, b2_nz = _prep_inputs(**inputs)
    nc = _build(b1_nz, b2_nz)
    res = bass_utils.run_bass_kernel_spmd(
        nc, in_maps, core_ids=list(range(NCORES)), trace=trace
    )
    # nll comes back [128, NBT] with [p, t] = row t*128+p
    out = np.concatenate(
        [r["nll"].T.reshape(-1) for r in res.results]
    ).astype(np.float32)
    return out, res


def kernel(**inputs) -> np.ndarray:
    out, _ = run(inputs, trace=False)
    return out


# revision 37
# speedup vs baseline: 1.4256x; 1.0415x over previous
"""Trainium2 Bass kernel for nn_ConditionalSoftmax (sampled-softmax NLL loss).

Computes, for each batch row b:
    v_c   = vectors[cs[b]]                      # [D]
    h     = relu(v_c @ W1 + b1)                 # [H]
    logit = h @ W2 + b2                         # [V]
    nll_b = logsumexp(logit) - logit[v2s[ws[b]]]

Sharding: data-parallel over batch across 8 NeuronCores (1024 rows/core),
weights replicated.  Per core the dominant work is the [1024,512]@[512,20000]
matmul plus the exp of all 20.5M logits.  The matmul runs in fp8_e4m3 with
the PE's DoubleRow perf mode (K=256 per instruction, 2x bf16 throughput);
W2 is pre-scaled by 32 on the host so its values sit in the fp8 normal
range, and the Exp activation's scale parameter undoes the factor for free.
W2 (fp8, 80KB/partition) stays resident in SBUF.  Logits accumulate in
[128,2000] PSUM tiles (4 banks, double buffered) and are reduced in place
by the ScalarEngine's fused exp+row-sum (accum_out), so the [1024,20000]
logit matrix never touches HBM and the per-instruction activation overhead
is amortized over 2000 columns.  The final log runs as ONE batched Ln over
[128,8] so the Exp/Ln activation tables swap exactly once.  The target
logit takes a separate cheap path: indirect-gather of the needed W2.T rows
(fp32) and a multiply-reduce on the VectorEngine against an fp32 recompute
of h.
"""

import numpy as np
import ml_dtypes

import concourse.bass as bass
import concourse.mybir as mybir
import concourse.tile as tile
from concourse import bacc, bass_utils
from concourse.bass import IndirectOffsetOnAxis, ts
from concourse.masks import make_identity

# Problem shapes (hardcoded per contest contract)
N_VOCAB = 50000
V = 20000
D = 300
DP = 384          # D padded to 3*128
NDC = 3           # contraction chunks for D
H = 512
NKG = 2           # DoubleRow contraction groups for H (256 each)
NHC = 4           # 128-row contraction chunks for H
B = 8192
NCORES = 8
BL = B // NCORES  # 1024 rows per core
NBT = BL // 128   # 8 batch tiles of 128 rows
VT = 1000         # vocab tile width (2 PSUM banks)
NVT = V // VT     # 20 vocab tiles
# matmul chunks within a VT tile (cannot cross a 512-f32 PSUM bank boundary)
VCHUNKS = ((0, 512), (512, VT - 512))

W2_SCALE = 32.0   # host pre-scale of W2 into fp8 range; undone by Exp scale

F32 = mybir.dt.float32
BF16 = mybir.dt.bfloat16
FP8 = mybir.dt.float8e4
I32 = mybir.dt.int32
AF = mybir.ActivationFunctionType
OP = mybir.AluOpType
DR = mybir.MatmulPerfMode.DoubleRow

_BUILD_CACHE = {}


def _build(b1_nz: bool, b2_nz: bool):
    key = (b1_nz, b2_nz)
    if key in _BUILD_CACHE:
        return _BUILD_CACHE[key]

    nc = bacc.Bacc(
        "TRN2",
        target_bir_lowering=False,
        debug=False,
        num_devices=NCORES,
        num_swdge_queues=2,
    )

    # Index tensors pre-transposed on the host to [128, NBT] so each loads
    # in ONE cheap DMA (contiguous 32B runs per partition).  ws is already
    # mapped through vector_to_support on the host.
    cs_idx = nc.dram_tensor("cs_idx", [128, NBT], I32, kind="ExternalInput").ap()
    ws_sup = nc.dram_tensor("ws_sup", [128, NBT], I32, kind="ExternalInput").ap()
    vectors = nc.dram_tensor("vectors", [N_VOCAB, D], F32, kind="ExternalInput").ap()
    w1 = nc.dram_tensor("w1", [DP, H], BF16, kind="ExternalInput").ap()
    b1c = nc.dram_tensor("b1c", [128, NHC], F32, kind="ExternalInput").ap()
    # W2 pre-scaled by W2_SCALE, fp8, laid out [v-chunk, p, kg, i, vt] with
    # W2s[kg*256 + i*128 + p, v*VT + vt] so each v-chunk is one fully
    # contiguous 1MB DRAM block (minimal DMA descriptor count — the early
    # window is descriptor-processing-bound across all 16 DMA queues).
    w2q = nc.dram_tensor(
        "w2q", [NVT, 128, NKG, 2, VT], FP8, kind="ExternalInput"
    ).ap()
    w2tb = nc.dram_tensor("w2tb", [V, H + 1], F32, kind="ExternalInput").ap()
    if b1_nz:
        b1rep = nc.dram_tensor("b1rep", [128, H], F32, kind="ExternalInput").ap()
    if b2_nz:
        b2rep = nc.dram_tensor("b2rep", [128, V], F32, kind="ExternalInput").ap()
    nll = nc.dram_tensor("nll", [128, NBT], F32, kind="ExternalOutput").ap()

    with tile.TileContext(nc) as tc:
        with (
            tc.tile_pool(name="consts", bufs=1) as consts,
            tc.tile_pool(name="idx", bufs=8) as idxp,
            tc.tile_pool(name="vc", bufs=8) as vcp,
            tc.tile_pool(name="gw", bufs=4) as gwp,
            tc.tile_pool(name="scr", bufs=3) as scrp,
            tc.tile_pool(name="acc", bufs=2) as accp,
            tc.tile_pool(name="ps", bufs=4, space="PSUM") as psm,
        ):
            # Early DMA issue order is the head latency: the Sync sequencer
            # issues serially (~600ns each) and the queues drain in FIFO
            # order.  The first phase-2 tile needs cidx (gather chain) and
            # W2 chunk 0, so those go first.
            cidx = consts.tile([128, NBT], I32)
            nc.sync.dma_start(cidx[:], cs_idx[:])
            widx = consts.tile([128, NBT], I32)
            nc.sync.dma_start(widx[:], ws_sup[:])

            # Resident fp8 W2, one tile per v-chunk so dependency tracking
            # is per-chunk and phase 2 starts as soon as chunk 0 lands.
            # Issued on Sync (NOT Scalar: the Scalar sequencer is in-order
            # and DMA flow-control waits there would stall every Exp).
            w2sbs = []
            for v in range(NVT):
                w2sbs.append(
                    consts.tile([128, NKG, 2, VT], FP8, name=f"w2sb{v}")
                )

            def load_w2(v):
                nc.sync.dma_start(w2sbs[v][:], w2q[v])

            # First two chunks ride gpsimd's software DGE, ahead of the
            # gathers in its stream: they land ~15us earlier than the Sync
            # HWDGE path, and the first sweep starts that much sooner.
            nc.gpsimd.dma_start(w2sbs[0][:], w2q[0])
            nc.gpsimd.dma_start(w2sbs[1][:], w2q[1])
            load_w2(2)
            load_w2(3)
            b1sb = consts.tile([128, NHC], F32)
            nc.sync.dma_start(b1sb[:], b1c[:])
            w1sb = consts.tile([128, NDC, H], BF16)
            nc.sync.dma_start(w1sb[:], w1.rearrange("(c p) h -> p c h", p=128))
            for v in range(4, NVT):
                load_w2(v)
            if b1_nz:
                b1rep_sb = consts.tile([128, H], F32)
                nc.sync.dma_start(b1rep_sb[:], b1rep[:])
            if b2_nz:
                b2rep_sb = consts.tile([128, V], F32)
                nc.sync.dma_start(b2rep_sb[:], b2rep[:])

            ident = consts.tile([128, 128], BF16)
            make_identity(nc, ident[:])

            # Long-lived activations
            vcT = consts.tile([128, NDC, BL], BF16)    # v_c^T, d-major
            hT8 = consts.tile([128, NKG, 2, BL], FP8)  # h^T fp8, DoubleRow layout
            hb = consts.tile([128, NBT, H], F32)       # h, batch-major (target dot)
            tdot = consts.tile([128, NBT], F32)        # target logits
            fin = consts.tile([128, 3 * NBT], F32)     # S | lnS | result

            # ---- Phase helpers, software-pipelined into the phase-2 sweep
            # so no engine's in-order stream ever bunches slow work in front
            # of the critical path. ----

            def gather(t):
                """vc embedding gather for batch tile t (gpsimd)."""
                vc = vcp.tile([128, D], F32, tag="vc", name=f"vc{t}")
                nc.gpsimd.indirect_dma_start(
                    out=vc[:],
                    out_offset=None,
                    in_=vectors[:],
                    in_offset=IndirectOffsetOnAxis(ap=cidx[:, t : t + 1], axis=0),
                )
                return vc

            def cast(t, vc):
                """bf16 cast for batch tile t."""
                vcb = vcp.tile([128, DP], BF16, tag="vcb", name=f"vcb{t}")
                nc.vector.memset(vcb[:, D:DP], 0.0)
                nc.vector.tensor_copy(vcb[:, :D], vc[:])
                return vcb

            def transpose(t, vcb):
                """PE transposes for batch tile t.  Kept off the DMA queues
                entirely: the W2 chunks keep the HWDGE rings busy for the
                first ~40us and any DMA behind them stalls.  Each transpose
                is its own accumulation group, so each goes to its own 2KB
                PSUM bank (1024 bf16 columns; a [128,2*VT] bf16 tile holds
                two banks, so the three d-chunks use two tiles)."""
                tp1 = psm.tile([128, 2 * VT], BF16, tag="ps", name=f"tp1_{t}")
                tp2 = psm.tile([128, 2 * VT], BF16, tag="ps", name=f"tp2_{t}")
                outs = [tp1[:, 0:128], tp1[:, 1024:1152], tp2[:, 0:128]]
                for c in range(NDC):
                    nc.tensor.transpose(outs[c], vcb[:, ts(c, 128)], ident[:])
                for c in range(NDC):
                    nc.vector.tensor_copy(vcT[:, c, ts(t, 128)], outs[c])

            def first_layer_hT(t, half):
                """W1 matmuls + relu + fp8 cast for batch tile t, two hc
                chunks per call.  Each hc chunk is an independent
                accumulation group, and start=True zeroes the whole 2KB
                PSUM bank, so each group gets its own bank."""
                pst = psm.tile([128, VT], F32, tag="ps", name=f"pst{t}_{half}")
                for i in range(2):
                    hc = 2 * half + i
                    for c in range(NDC):
                        nc.tensor.matmul(
                            pst[:, i * 512 : i * 512 + 128],
                            lhsT=w1sb[:, c, ts(hc, 128)],
                            rhs=vcT[:, c, ts(t, 128)],
                            start=(c == 0),
                            stop=(c == NDC - 1),
                        )
                # relu + bias, cast to fp8 DoubleRow layout (DVE)
                for i in range(2):
                    hc = 2 * half + i
                    nc.vector.tensor_scalar(
                        out=hT8[:, hc // 2, hc % 2, ts(t, 128)],
                        in0=pst[:, i * 512 : i * 512 + 128],
                        scalar1=b1sb[:, hc : hc + 1],
                        scalar2=0.0,
                        op0=OP.add,
                        op1=OP.max,
                    )

            def first_layer_hb(t):
                """Batch-major fp32 h for the target-logit dot."""
                psb = psm.tile([128, VT], F32, tag="ps", name=f"psb{t}")
                for c in range(NDC):
                    nc.tensor.matmul(
                        psb[:, :512],
                        lhsT=vcT[:, c, ts(t, 128)],
                        rhs=w1sb[:, c, :],
                        start=(c == 0),
                        stop=(c == NDC - 1),
                    )
                if b1_nz:
                    nc.vector.tensor_add(psb[:, :512], psb[:, :512], b1rep_sb[:])
                nc.vector.tensor_scalar_max(hb[:, t, :], psb[:, :512], 0.0)

            def target_dot(t):
                """Gather W2.T row of the target, fp32 dot with h."""
                g = gwp.tile([128, H + 1], F32, tag="g", name=f"g{t}")
                nc.gpsimd.indirect_dma_start(
                    out=g[:],
                    out_offset=None,
                    in_=w2tb[:],
                    in_offset=IndirectOffsetOnAxis(ap=widx[:, t : t + 1], axis=0),
                )
                # (tensor_tensor_reduce is broken on this HW path; use 3 ops)
                gscr = gwp.tile([128, H], F32, tag="gscr", name=f"gscr{t}")
                nc.vector.tensor_mul(gscr[:], hb[:, t, :], g[:, :H])
                gacc = gwp.tile([128, 1], F32, tag="gacc", name=f"gacc{t}")
                nc.vector.reduce_sum(
                    out=gacc[:], in_=gscr[:], axis=mybir.AxisListType.X
                )
                nc.vector.tensor_add(tdot[:, t : t + 1], gacc[:], g[:, H : H + 1])

            # ---- Main pipeline.  All gathers queue on gpsimd immediately
            # (they pace at several us each due to SWDGE ring flow control,
            # but nothing downstream needs tile t before its sweep).  Per
            # batch-tile sweep over v, the NEXT tile's cast/transpose,
            # first-layer and this tile's target-dot are injected at
            # staggered points so no in-order engine stream ever queues
            # slow work in front of critical work.  The exp row-sums run on
            # the otherwise-idle VectorEngine (bf16, 2x mode) so the Scalar
            # engine does nothing but back-to-back Exp. ----
            vcs = [gather(t) for t in range(NBT)]
            transpose(0, cast(0, vcs[0]))
            first_layer_hT(0, 0)
            first_layer_hT(0, 1)
            first_layer_hb(0)
            vcbs = {}
            for t in range(NBT):
                acc = accp.tile([128, VT], BF16, tag="acc", name=f"acc{t}")
                for v in range(NVT):
                    if v == 4 and t + 1 < NBT:
                        vcbs[t + 1] = cast(t + 1, vcs[t + 1])
                    if v == 8 and t + 1 < NBT:
                        transpose(t + 1, vcbs[t + 1])
                    if v == 12 and t + 1 < NBT:
                        first_layer_hT(t + 1, 0)
                    if v == 13 and t + 1 < NBT:
                        first_layer_hT(t + 1, 1)
                    if v == 15 and t + 1 < NBT:
                        first_layer_hb(t + 1)
                    if v == 17:
                        target_dot(t)
                    ps = psm.tile([128, VT], F32, tag="ps", name=f"ps{t}_{v}")
                    for lo, w in VCHUNKS:
                        for kg in range(NKG):
                            nc.tensor.matmul(
                                ps[:, lo : lo + w],
                                lhsT=hT8[:, kg, :, ts(t, 128)],
                                rhs=w2sbs[v][:, kg, :, lo : lo + w],
                                start=(kg == 0),
                                stop=(kg == NKG - 1),
                                perf_mode=DR,
                            )
                    if b2_nz:
                        nc.vector.tensor_add(
                            ps[:], ps[:], b2rep_sb[:, ts(v, VT)]
                        )
                    escr = scrp.tile([128, VT], BF16, tag="escr", name=f"e{t}_{v}")
                    nc.scalar.activation(
                        escr[:], ps[:], AF.Exp, scale=1.0 / W2_SCALE
                    )
                    if v == 0:
                        nc.vector.tensor_copy(acc[:], escr[:])
                    else:
                        nc.vector.tensor_add(acc[:], acc[:], escr[:])
                nc.vector.reduce_sum(
                    out=fin[:, t : t + 1], in_=acc[:], axis=mybir.AxisListType.X
                )

            # ---- Phase 3: logsumexp and output.  One batched Ln so the
            # Exp->Ln activation-table swap happens exactly once. ----
            nc.scalar.activation(fin[:, NBT : 2 * NBT], fin[:, :NBT], AF.Ln)
            nc.vector.tensor_sub(
                fin[:, 2 * NBT : 3 * NBT], fin[:, NBT : 2 * NBT], tdot[:, :NBT]
            )
            nc.sync.dma_start(nll[:], fin[:, 2 * NBT : 3 * NBT])

    nc.compile()
    _BUILD_CACHE[key] = nc
    return nc


def _prep_inputs(ws, cs, vectors, W1, b1, W2, b2, vector_to_support):
    ws = np.asarray(ws)
    cs = np.asarray(cs)
    vectors = np.asarray(vectors, dtype=np.float32)
    W1 = np.asarray(W1, dtype=np.float32)
    b1 = np.asarray(b1, dtype=np.float32)
    W2 = np.asarray(W2, dtype=np.float32)
    b2 = np.asarray(b2, dtype=np.float32)
    v2s = np.asarray(vector_to_support)

    b1_nz = bool(np.any(b1))
    b2_nz = bool(np.any(b2))

    w1p = np.zeros((DP, H), dtype=ml_dtypes.bfloat16)
    w1p[:D] = W1.astype(ml_dtypes.bfloat16)
    # fp8 DoubleRow layout, v-chunk-major per partition:
    # w2q[p, v, kg, i, vt] = (W2*S)[kg*256 + i*128 + p, v*VT + vt]
    w2s = (W2 * W2_SCALE).astype(ml_dtypes.float8_e4m3)
    w2q = np.ascontiguousarray(
        w2s.reshape(NKG, 2, 128, NVT, VT).transpose(3, 2, 0, 1, 4)
    )
    w2tb = np.ascontiguousarray(
        np.concatenate([W2.T, b2[:, None]], axis=1).astype(np.float32)
    )
    b1c = np.ascontiguousarray(b1.reshape(NHC, 128).T)

    shared = {
        "vectors": np.ascontiguousarray(vectors),
        "w1": w1p,
        "b1c": b1c,
        "w2q": w2q,
        "w2tb": w2tb,
    }
    if b1_nz:
        shared["b1rep"] = np.ascontiguousarray(
            np.broadcast_to(b1, (128, H)).astype(np.float32)
        )
    if b2_nz:
        shared["b2rep"] = np.ascontiguousarray(
            np.broadcast_to(b2, (128, V)).astype(np.float32)
        )

    # map target word -> support index on the host (pure input indexing)
    ws_sup = v2s[np.asarray(ws)].astype(np.int32)

    in_maps = []
    for c in range(NCORES):
        sl = slice(c * BL, (c + 1) * BL)
        m = dict(shared)
        # host-transposed to [128, NBT]: [p, t] = idx[t*128 + p]
        m["cs_idx"] = np.ascontiguousarray(
            cs[sl].astype(np.int32).reshape(NBT, 128).T
        )
        m["ws_sup"] = np.ascontiguousarray(
            ws_sup[sl].reshape(NBT, 128).T
        )
        in_maps.append(m)
    return in_maps, b1_nz, b2_nz


def run(inputs: dict, trace: bool = False):
    """Run the SPMD kernel. Returns (output [B] fp32, BassKernelResults)."""
    in_maps, b1_nz, b2_nz = _prep_inputs(**inputs)
    nc = _build(b1_nz, b2_nz)
    res = bass_utils.run_bass_kernel_spmd(
        nc, in_maps, core_ids=list(range(NCORES)), trace=trace
    )
    # nll comes back [128, NBT] with [p, t] = row t*128+p
    out = np.concatenate(
        [r["nll"].T.reshape(-1) for r in res.results]
    ).astype(np.float32)
    return out, res


def kernel(**inputs) -> np.ndarray:
    out, _ = run(inputs, trace=False)
    return out
